# revision 14
# baseline (speedup 1.0000x reference)
"""Trainium2 Bass kernel for a DGCNN-style point-cloud encoder.

Per batch element (one per NeuronCore, B=8): kNN graph (k=20) over N=4096
points via a distance matmul + iterative top-8 extraction (max/max_index/
match_replace), edge-feature MLP with two training-mode batchnorms (global
stats via cross-core AllReduce) and leaky-relu, then max-pool over
neighbors.  Layout strategy: the first MLP layer is decomposed into
per-point projections A = W1a@p and C = (W1b-W1a)@p; the gather of A rows
by neighbor index runs as an indirect DMA with compute_op=add onto a
C-prefilled tile, so h1_pre arrives in one pass; PE transposes pairs of
neighbors into channelx2 PSUM tiles for the W2 stage.
"""
import sys
sys.path.insert(0, '/opt/trn_rl_repo')

import numpy as np
import orjson

import concourse.bass as bass
import concourse.mybir as mybir
import concourse.tile as tile
from concourse.bass_utils import run_bass_kernel_spmd
from concourse.masks import make_identity

# ---------------------------------------------------------------------------
# Workaround for walrus 'Too many sync wait commands': this toolchain accepts
# at most one sem-wait per lowered instruction. Split any instruction carrying
# more waits into EventSemaphore wait-carriers placed immediately before it.
# ---------------------------------------------------------------------------
_MAXW = 1


def _split_excess_waits(j) -> bool:
    changed = False
    for fn in j.get("functions", []):
        for blk in fn.get("blocks", []):
            out = []
            for inst in blk.get("instructions", []):
                si = inst.get("sync_info") or {}
                ow = si.get("on_wait") or []
                if len(ow) > _MAXW:
                    changed = True
                    chunks = [ow[i:i + _MAXW] for i in range(0, len(ow), _MAXW)]
                    for ci, chunk in enumerate(chunks[:-1]):
                        out.append({
                            "debug": inst.get("debug", 0),
                            "engine": inst["engine"],
                            "ins": [], "outs": [],
                            "name": f"{inst['name']}-w{ci}",
                            "opcode": "EventSemaphore",
                            "sync_info": {"on_update": [], "on_wait": chunk},
                        })
                    si = dict(si)
                    si["on_wait"] = chunks[-1]
                    inst = dict(inst)
                    inst["sync_info"] = si
                out.append(inst)
            blk["instructions"] = out
    return changed


_orig_to_json_bytes = bass.Bass.to_json_bytes


def _patched_to_json_bytes(self) -> bytes:
    raw = _orig_to_json_bytes(self)
    j = orjson.loads(raw)
    if _split_excess_waits(j):
        return orjson.dumps(j)
    return raw


bass.Bass.to_json_bytes = _patched_to_json_bytes

# ---------------------------------------------------------------------------
# Problem constants (hardcoded; kernel.py must be self-contained)
# ---------------------------------------------------------------------------
B = 8            # batch = number of cores
N = 4096         # points per cloud
KNN = 20         # neighbors
CH = 64          # hidden channels
EPS = 1e-5
ALPHA = 0.2      # leaky-relu slope
NM = N // 128    # 32 row-tiles
NJ2 = KNN // 2   # 10 neighbor pairs
CNT = B * N * KNN  # batchnorm population size (global over all cores)
NEG = -1.0e30
PGROUPS = [(0, 4), (4, 8), (8, 10)]     # j2 pair-groups per psum tile
WCHUNKS = [(0, 512), (512, 1024), (1024, 1280)]  # W2 rhs chunks per parity

f32 = mybir.dt.float32
f16 = mybir.dt.float16
u32 = mybir.dt.uint32
ACTF = mybir.ActivationFunctionType


def _bcast_mid(ap, reps):
    """Insert a step-0 dim after the partition dim: [P, F] -> [P, reps, F]."""
    return bass.AP(ap.tensor, ap.offset,
                   [list(ap.ap[0]), [0, reps], list(ap.ap[1])])


def _build_program():
    nc = bass.Bass("TRN2", target_bir_lowering=False, debug=False,
                   num_devices=B)

    xb = nc.dram_tensor("xb", [4, N], f32, kind="ExternalInput")
    w1at = nc.dram_tensor("w1at", [3, CH], f32, kind="ExternalInput")
    w1ct = nc.dram_tensor("w1ct", [3, CH], f32, kind="ExternalInput")
    w2t = nc.dram_tensor("w2t", [128, CH], f16, kind="ExternalInput")
    bn1g = nc.dram_tensor("bn1g", [CH, 1], f32, kind="ExternalInput")
    bn1b = nc.dram_tensor("bn1b", [CH, 1], f32, kind="ExternalInput")
    bn2g = nc.dram_tensor("bn2g", [CH, 1], f32, kind="ExternalInput")
    bn2b = nc.dram_tensor("bn2b", [CH, 1], f32, kind="ExternalInput")
    out_t = nc.dram_tensor("out", [CH, N], f32, kind="ExternalOutput")

    cc1_in = nc.dram_tensor("cc1_in", [128, 2], f32)
    cc1_out = nc.dram_tensor("cc1_out", [128, 2], f32, addr_space="Shared")
    cc2_in = nc.dram_tensor("cc2_in", [CH, 2], f32)
    cc2_out = nc.dram_tensor("cc2_out", [CH, 2], f32, addr_space="Shared")
    groups = [list(range(B))]

    with tile.TileContext(nc) as tc:
        const = tc.alloc_tile_pool(name="const", bufs=1)
        dramp = tc.alloc_tile_pool(name="dram", bufs=1, space="DRAM")
        abpool = tc.alloc_tile_pool(name="ab", bufs=1)

        # whole-kernel tensors
        w2t_sb = const.tile([128, CH], f16)
        g1_sb = const.tile([CH, 1], f32)
        b1in_sb = const.tile([CH, 1], f32)
        g2_sb = const.tile([CH, 1], f32)
        b2in_sb = const.tile([CH, 1], f32)
        h1p = const.tile([128, NM * NJ2 * 128], f16)   # stored h1_pre
        a1_sb = const.tile([128, 1], f32)
        b1_sb = const.tile([128, 1], f32)
        a2_sb = const.tile([CH, 1], f32)
        b2_sb = const.tile([CH, 1], f32)
        s2st6 = const.tile([CH, NM * 6 * 6], f32)      # bn_stats slots (C)

        # phase A/B tensors (released after phase AB)
        lhs_all = abpool.tile([4, N], f32)      # [x;y;z;1]
        rhs_all = abpool.tile([4, N], f32)      # [2x;2y;2z;-|p|^2]
        ct_sb = abpool.tile([128, CH * NM], f32)  # C features, [i, c] blocks
        ident = abpool.tile([128, 128], f32)
        w1at_sb = abpool.tile([3, CH], f32)
        w1ct_sb = abpool.tile([3, CH], f32)
        s1sum = abpool.tile([128, 3 * NM], f32)
        s1sq = abpool.tile([128, 3 * NM], f32)

        at_dram = dramp.tile([N, CH], f32)

        nc.sync.dma_start(lhs_all[:], xb.ap())
        nc.sync.dma_start(w1at_sb[:], w1at.ap())
        nc.sync.dma_start(w1ct_sb[:], w1ct.ap())
        nc.sync.dma_start(w2t_sb[:], w2t.ap())
        nc.sync.dma_start(g1_sb[:], bn1g.ap())
        nc.sync.dma_start(b1in_sb[:], bn1b.ap())
        nc.sync.dma_start(g2_sb[:], bn2g.ap())
        nc.sync.dma_start(b2in_sb[:], bn2b.ap())
        make_identity(nc, ident)

        with nc.named_scope("stage0"):
            with tc.tile_pool(name="s0", bufs=2) as s0pool, \
                 tc.tile_pool(name="s0ps", bufs=2, space="PSUM") as s0psum:
                nc.scalar.mul(rhs_all[0:3, :], lhs_all[0:3, :], 2.0)
                xsq = s0pool.tile([3, N], f32)
                nc.scalar.square(xsq[:], lhs_all[0:3, :])
                ones3 = nc.const_aps.tensor(1.0, (3, 1), f32)
                for j in range(N // 512):
                    ps = s0psum.tile([1, 512], f32, space="PSUM", tag="sq")
                    nc.tensor.matmul(ps[:], lhsT=ones3,
                                     rhs=xsq[:, bass.ts(j, 512)],
                                     start=True, stop=True)
                    sqneg = s0pool.tile([1, 512], f32, tag="sqneg")
                    nc.scalar.activation(sqneg[:], ps[:], ACTF.Copy, scale=-1.0)
                    nc.sync.dma_start(rhs_all[3:4, bass.ts(j, 512)], sqneg[:])
                # per-point projections: AT rows (gather table) and C features
                for m in range(NM):
                    ap_ = s0psum.tile([128, CH], f32, space="PSUM", tag="at")
                    nc.tensor.matmul(ap_[:], lhsT=lhs_all[0:3, bass.ts(m, 128)],
                                     rhs=w1at_sb[:], start=True, stop=True)
                    at_sb = s0pool.tile([128, CH], f32, tag="atsb")
                    nc.scalar.copy(at_sb[:], ap_[:])
                    nc.sync.dma_start(at_dram[m * 128:(m + 1) * 128, :], at_sb[:])
                    cp_ = s0psum.tile([128, CH], f32, space="PSUM", tag="ct")
                    nc.tensor.matmul(cp_[:], lhsT=lhs_all[0:3, bass.ts(m, 128)],
                                     rhs=w1ct_sb[:], start=True, stop=True)
                    nc.scalar.copy(ct_sb[:, bass.ts(m, CH)], cp_[:])

        # ------------------------------------------------------------------
        # Phases A (scores + top-k) and B (gather+add, transpose, BN1 stats)
        # ------------------------------------------------------------------
        with nc.named_scope("phaseAB"), \
             tc.tile_pool(name="scps", bufs=5, space="PSUM") as scps, \
             tc.tile_pool(name="score", bufs=3) as scorep, \
             tc.tile_pool(name="idxp", bufs=4) as idxp, \
             tc.tile_pool(name="gat", bufs=2) as gatp, \
             tc.tile_pool(name="trps", bufs=3, space="PSUM") as trps, \
             tc.tile_pool(name="dmy", bufs=2) as dmyp:
            def emit_scores(m):
                score = scorep.tile([128, N], f32, tag="score")
                for j in range(N // 512):
                    ps = scps.tile([128, 512], f32, space="PSUM", tag="sc")
                    nc.tensor.matmul(ps[:], lhsT=lhs_all[:, bass.ts(m, 128)],
                                     rhs=rhs_all[:, bass.ts(j, 512)],
                                     start=True, stop=True)
                    nc.scalar.copy(score[:, bass.ts(j, 512)], ps[:])
                return score

            for m in range(NM):
                score = emit_scores(m)
                idx24 = idxp.tile([128, 24], u32, tag="idx")
                for r in range(3):
                    vals = idxp.tile([128, 8], f32, tag="vals")
                    nc.vector.max(out=vals[:], in_=score[:])
                    nc.vector.max_index(out=idx24[:, r * 8:(r + 1) * 8],
                                        in_max=vals[:], in_values=score[:])
                    if r < 2:
                        nc.vector.match_replace(out=score[:], in_to_replace=vals[:],
                                                in_values=score[:], imm_value=NEG)
                # gather h1_pre = A[idx] + C  (C prefilled, DMA compute add)
                gat = gatp.tile([128, KNN * CH], f32, tag="gat")
                nc.sync.dma_start(
                    gat[:].rearrange("p (k c) -> p k c", k=KNN),
                    _bcast_mid(ct_sb[:, bass.ts(m, CH)], KNN))
                for kk in range(KNN):
                    nc.gpsimd.indirect_dma_start(
                        out=gat[:, kk * CH:(kk + 1) * CH], out_offset=None,
                        in_=at_dram[:],
                        in_offset=bass.IndirectOffsetOnAxis(
                            ap=idx24[:, kk:kk + 1], axis=0),
                        compute_op=mybir.AluOpType.add)
                for gi, (j2a, j2b) in enumerate(PGROUPS):
                    gw = (j2b - j2a) * 128
                    trp = trps.tile([128, 512], f32, space="PSUM", tag="tr")
                    for g in range(j2b - j2a):
                        nc.tensor.matmul(
                            trp[:, g * 128:(g + 1) * 128],
                            lhsT=gat[:, (j2a + g) * 128:(j2a + g + 1) * 128],
                            rhs=ident[:], is_transpose=True,
                            start=True, stop=True)
                    col = (m * NJ2 + j2a) * 128
                    scol = m * 3 + gi
                    nc.scalar.activation(h1p[:, col:col + gw], trp[:, 0:gw],
                                         ACTF.Copy,
                                         accum_out=s1sum[:, scol:scol + 1])
                    dmy = dmyp.tile([128, 512], f32, tag="dmy")
                    nc.scalar.activation(dmy[:, 0:gw], trp[:, 0:gw],
                                         ACTF.Square,
                                         accum_out=s1sq[:, scol:scol + 1])

        # ------------------------------------------------------------------
        # BN1: global stats -> a1, b1
        # ------------------------------------------------------------------
        with nc.named_scope("bn1"), tc.tile_pool(name="bn1p", bufs=1) as bnp:
            st1 = bnp.tile([128, 2], f32)
            nc.vector.tensor_reduce(out=st1[:, 0:1], in_=s1sum[:],
                                    axis=mybir.AxisListType.X,
                                    op=mybir.AluOpType.add)
            nc.vector.tensor_reduce(out=st1[:, 1:2], in_=s1sq[:],
                                    axis=mybir.AxisListType.X,
                                    op=mybir.AluOpType.add)
            nc.sync.dma_start(cc1_in.ap(), st1[:])
            nc.gpsimd.collective_compute(
                kind="AllReduce", op=mybir.AluOpType.add,
                replica_groups=groups, ins=[cc1_in.ap()], outs=[cc1_out.ap()])
            st1g = bnp.tile([128, 2], f32)
            nc.sync.dma_start(st1g[:], cc1_out.ap())
            st1hi = bnp.tile([CH, 2], f32)
            nc.sync.dma_start(st1hi[:], st1g[CH:128, :])
            tot1 = bnp.tile([CH, 2], f32)
            nc.vector.tensor_add(tot1[:], st1g[0:CH, :], st1hi[:])
            mean1 = bnp.tile([CH, 1], f32)
            nc.scalar.mul(mean1[:], tot1[:, 0:1], 1.0 / CNT)
            ex2 = bnp.tile([CH, 1], f32)
            nc.scalar.mul(ex2[:], tot1[:, 1:2], 1.0 / CNT)
            msq = bnp.tile([CH, 1], f32)
            nc.scalar.square(msq[:], mean1[:])
            var1 = bnp.tile([CH, 1], f32)
            nc.vector.tensor_sub(var1[:], ex2[:], msq[:])
            nc.scalar.activation(var1[:], var1[:], ACTF.Copy, bias=EPS)
            rcp1 = bnp.tile([CH, 1], f32)
            nc.vector.reciprocal(rcp1[:], var1[:])
            rs1 = bnp.tile([CH, 1], f32)
            nc.scalar.sqrt(rs1[:], rcp1[:])
            a1h = bnp.tile([CH, 1], f32)
            nc.vector.tensor_mul(a1h[:], rs1[:], g1_sb[:])
            am = bnp.tile([CH, 1], f32)
            nc.vector.tensor_mul(am[:], a1h[:], mean1[:])
            b1h = bnp.tile([CH, 1], f32)
            nc.vector.tensor_sub(b1h[:], b1in_sb[:], am[:])
            nc.sync.dma_start(a1_sb[0:CH, :], a1h[:])
            nc.sync.dma_start(a1_sb[CH:128, :], a1h[:])
            nc.sync.dma_start(b1_sb[0:CH, :], b1h[:])
            nc.sync.dma_start(b1_sb[CH:128, :], b1h[:])

        # ------------------------------------------------------------------
        # Phase C: h2_pre = W2 @ lrelu(a1*h1_pre + b1); BN2 stats (bn_stats)
        # ------------------------------------------------------------------
        with nc.named_scope("phaseC"), \
             tc.tile_pool(name="h1a", bufs=2) as h1ap, \
             tc.tile_pool(name="h2ps", bufs=6, space="PSUM") as h2ps:
            for m in range(NM):
                mcol = m * NJ2 * 128
                h1a = h1ap.tile([128, NJ2 * 128], f16, tag="h1a")
                nc.scalar.activation(h1a[:], h1p[:, mcol:mcol + NJ2 * 128],
                                     ACTF.Prelu, bias=b1_sb[:, 0:1],
                                     scale=a1_sb[:, 0:1], alpha=ALPHA)
                for par in range(2):
                    for ci, (c0, c1) in enumerate(WCHUNKS):
                        cw = c1 - c0
                        hp = h2ps.tile([CH, 512], f32, space="PSUM", tag="h2")
                        nc.tensor.matmul(
                            hp[:, 0:cw],
                            lhsT=w2t_sb[par * CH:(par + 1) * CH, :],
                            rhs=h1a[par * CH:(par + 1) * CH, c0:c1],
                            start=True, stop=True)
                        scol = ((m * 2 + par) * 3 + ci) * 6
                        nc.vector.bn_stats(s2st6[:, scol:scol + 6], hp[:, 0:cw])

        # ------------------------------------------------------------------
        # BN2: aggregate + global stats -> a2, b2
        # ------------------------------------------------------------------
        with nc.named_scope("bn2"), tc.tile_pool(name="bn2p", bufs=1) as bnp:
            agg = bnp.tile([CH, 2], f32)
            nc.vector.bn_aggr(agg[:],
                              s2st6[:].rearrange("p (n s) -> p n s", s=6))
            msq2 = bnp.tile([CH, 1], f32)
            nc.scalar.square(msq2[:], agg[:, 0:1])
            st2 = bnp.tile([CH, 2], f32)
            nc.vector.tensor_copy(st2[:, 0:1], agg[:, 0:1])
            nc.vector.tensor_add(st2[:, 1:2], agg[:, 1:2], msq2[:])
            nc.sync.dma_start(cc2_in.ap(), st2[:])
            nc.gpsimd.collective_compute(
                kind="AllReduce", op=mybir.AluOpType.add,
                replica_groups=groups, ins=[cc2_in.ap()], outs=[cc2_out.ap()])
            tot2 = bnp.tile([CH, 2], f32)
            nc.sync.dma_start(tot2[:], cc2_out.ap())
            mean2 = bnp.tile([CH, 1], f32)
            nc.scalar.mul(mean2[:], tot2[:, 0:1], 1.0 / B)
            ex2b = bnp.tile([CH, 1], f32)
            nc.scalar.mul(ex2b[:], tot2[:, 1:2], 1.0 / B)
            msq2b = bnp.tile([CH, 1], f32)
            nc.scalar.square(msq2b[:], mean2[:])
            var2 = bnp.tile([CH, 1], f32)
            nc.vector.tensor_sub(var2[:], ex2b[:], msq2b[:])
            nc.scalar.activation(var2[:], var2[:], ACTF.Copy, bias=EPS)
            rcp2 = bnp.tile([CH, 1], f32)
            nc.vector.reciprocal(rcp2[:], var2[:])
            rs2 = bnp.tile([CH, 1], f32)
            nc.scalar.sqrt(rs2[:], rcp2[:])
            nc.vector.tensor_mul(a2_sb[:], rs2[:], g2_sb[:])
            am2 = bnp.tile([CH, 1], f32)
            nc.vector.tensor_mul(am2[:], a2_sb[:], mean2[:])
            nc.vector.tensor_sub(b2_sb[:], b2in_sb[:], am2[:])

        # ------------------------------------------------------------------
        # Phase D: recompute h2, apply BN2 + lrelu, max-pool over neighbors
        # ------------------------------------------------------------------
        # lrelu is monotone and the host folds sign(gamma2) into W2 (and
        # feeds |gamma2|), so a2 >= 0 and BN2-apply + lrelu commute with the
        # neighbor max-pool: pool raw h2_pre from PSUM, then one Prelu.
        with nc.named_scope("phaseD"), \
             tc.tile_pool(name="h1b", bufs=2) as h1bp, \
             tc.tile_pool(name="h2psd", bufs=6, space="PSUM") as h2psd, \
             tc.tile_pool(name="pmx", bufs=2) as pmxp, \
             tc.tile_pool(name="runm", bufs=2) as runp:
            for m in range(NM):
                mcol = m * NJ2 * 128
                h1a = h1bp.tile([128, NJ2 * 128], f16, tag="h1b")
                nc.scalar.activation(h1a[:], h1p[:, mcol:mcol + NJ2 * 128],
                                     ACTF.Prelu, bias=b1_sb[:, 0:1],
                                     scale=a1_sb[:, 0:1], alpha=ALPHA)
                pmax = pmxp.tile([CH, 6 * 128], f32, tag="pmx")
                for par in range(2):
                    for ci, (c0, c1) in enumerate(WCHUNKS):
                        cw = c1 - c0
                        nj = cw // 128
                        hp = h2psd.tile([CH, 512], f32, space="PSUM", tag="h2d")
                        nc.tensor.matmul(
                            hp[:, 0:cw],
                            lhsT=w2t_sb[par * CH:(par + 1) * CH, :],
                            rhs=h1a[par * CH:(par + 1) * CH, c0:c1],
                            start=True, stop=True)
                        slot = (par * 3 + ci) * 128
                        hb = hp[:]
                        pm_in = bass.AP(hb.tensor, hb.offset,
                                        [list(hb.ap[0]), [1, 128], [128, nj]])
                        nc.vector.tensor_reduce(
                            out=pmax[:, slot:slot + 128], in_=pm_in,
                            axis=mybir.AxisListType.X, op=mybir.AluOpType.max)
                pooled = runp.tile([CH, 128], f32, tag="pool")
                pb = pmax[:]
                rm_in = bass.AP(pb.tensor, pb.offset,
                                [list(pb.ap[0]), [1, 128], [128, 6]])
                nc.vector.tensor_reduce(
                    out=pooled[:], in_=rm_in,
                    axis=mybir.AxisListType.X, op=mybir.AluOpType.max)
                runmax = runp.tile([CH, 128], f32, tag="run")
                nc.scalar.activation(runmax[:], pooled[:],
                                     ACTF.Prelu, bias=b2_sb[:, 0:1],
                                     scale=a2_sb[:, 0:1], alpha=ALPHA)
                nc.sync.dma_start(out_t.ap()[:, bass.ts(m, 128)], runmax[:])

        abpool.release()
        const.release()
        dramp.release()

    return nc


_prog_cache = {}


def _get_program():
    if "nc" not in _prog_cache:
        _prog_cache["nc"] = _build_program()
    return _prog_cache["nc"]


def kernel(x, W1, gamma1, beta1, W2, gamma2, beta2):
    x = np.asarray(x, dtype=np.float32)
    W1 = np.asarray(W1, dtype=np.float32)
    W2 = np.asarray(W2, dtype=np.float32)
    w1at = np.ascontiguousarray(W1[:, 0:3].T)                 # [3, 64]
    w1ct = np.ascontiguousarray((W1[:, 3:6] - W1[:, 0:3]).T)  # [3, 64]
    g2 = np.asarray(gamma2, dtype=np.float32).reshape(CH)
    sgn2 = np.where(g2 < 0, -1.0, 1.0).astype(np.float32)
    W2f = W2 * sgn2[:, None]          # flip rows so the BN2 scale is >= 0
    w2t_1 = np.ascontiguousarray(W2f.T).astype(np.float16)    # [64, 64]
    w2t = np.concatenate([w2t_1, w2t_1], axis=0)              # [128, 64]
    col = lambda v: np.ascontiguousarray(
        np.asarray(v, dtype=np.float32).reshape(CH, 1))

    nc = _get_program()
    in_maps = []
    for b in range(B):
        in_maps.append({
            "xb": np.concatenate([x[b], np.ones((1, N), np.float32)], axis=0),
            "w1at": w1at, "w1ct": w1ct, "w2t": w2t,
            "bn1g": col(gamma1), "bn1b": col(beta1),
            "bn2g": col(np.abs(g2)), "bn2b": col(beta2),
        })
    res = run_bass_kernel_spmd(nc, in_maps, list(range(B)))
    out = np.stack([res.results[b]["out"] for b in range(B)], axis=0)
    return out.astype(np.float32)
